# revision 8
# baseline (speedup 1.0000x reference)
"""Trainium2 Bass kernel for the per-gene sparse-decoder MLP.

Math (per gene g): h = selu(features[:, tf_idx[g]] @ W1[g].T); h = selu(h @ Wm[i,g].T) x2;
out[:, g] = h @ Wf[g].  Genes are independent -> shard G=20000 over 8 cores (2500 each).

Device mapping per core (gene dim padded 2500 -> 2560):
  - Activations live as [node-rows on partitions, batch on free].  Per "unit" of
    32 genes: L1 is 4 matmuls of [128c=(8 genes x 16k), M=64=(8g x 8w), N=256]
    with block-diagonal lhsT; L2/L3/Lf are per-16-gene-group matmul PAIRS sharing
    one block-diagonal lhsT, accumulating two SELU streams into PSUM:
        selu(z) = A + B,  A = lam*relu(z)          (ACT Relu or DVE tensor_scalar)
                          B = min(lam*alp*e^z, lam*alp) - lam*alp
                            (ACT Exp(z + ln(lam*alp)) then DVE min/add)
    using exp(min(z,0)) == min(exp(z), 1) so no extra PSUM pass is needed.
  - The first-layer gather features[:, tf_idx] is compile-time data movement; it
    is laid out on the host into the exact [pack, row, batch] bf16 tiles the
    TensorEngine streams (this toolchain has no HIPI ucode, so the on-device
    dma_gather instruction is unavailable; the device still reads every gathered
    byte from HBM either way).
"""

import sys
import numpy as np

if "/opt/trn_rl_repo" not in sys.path:
    sys.path.insert(0, "/opt/trn_rl_repo")

import ml_dtypes

BF16 = ml_dtypes.bfloat16

G, W, K, T, B, D = 20000, 8, 16, 1500, 256, 2
NCORES = 8
GC = G // NCORES            # 2500 genes per core
GP = 2560                   # padded genes per core
NP = GP // 8                # 320 L1 packs
NG = GP // 16               # 160 groups
NU = GP // 32               # 80 units
NW = NU // 2                # 40 output windows (64 genes each)
NS = NU // 8                # 10 supers (8 units each)

LAM = 1.0507009873554805
ALPHA = 1.6732632423543772
LA = LAM * ALPHA
C0 = float(np.log(LA))      # Exp bias: exp(z + C0) = LA * e^z

_CACHE = {}
_DISABLE = set()   # debug: subsets of {'act','dve','evac'}


def _build():
    import concourse.bass as bass
    import concourse.mybir as mybir

    f32 = mybir.dt.float32
    bf16 = mybir.dt.bfloat16
    Alu = mybir.AluOpType
    Act = mybir.ActivationFunctionType

    nc = bass.Bass()

    def reg_const(value, dtype=f32):
        t = nc.alloc_sbuf_tensor(f"const-{dtype.name}-{value}", [128, 1], dtype)
        nc.gpsimd.memset(t.ap(), value)
        nc.const_aps.aps[(dtype, value)] = t.ap()

    reg_const(C0)
    nc.all_engine_barrier()

    xg_d = nc.declare_dram_parameter("xg", [NS, 128, 32, 256], bf16, isOutput=False)
    w1_d = nc.declare_dram_parameter("w1", [NS, 128, 32, 64], bf16, isOutput=False)
    wm2_d = nc.declare_dram_parameter("wm2", [NS, 128, 16, 128], bf16, isOutput=False)
    wm3_d = nc.declare_dram_parameter("wm3", [NS, 128, 16, 128], bf16, isOutput=False)
    wf_d = nc.declare_dram_parameter("wf", [NS, 128, 16, 16], bf16, isOutput=False)
    out_d = nc.declare_dram_parameter("out", [NW, 128, 256], f32, isOutput=True)

    from contextlib import ExitStack
    with ExitStack() as ctx:
        block = ctx.enter_context(nc.Block())
        def sb(name, shape, dt=bf16):
            return ctx.enter_context(nc.sbuf_tensor(name, shape, dt))
        def ps(name):
            return ctx.enter_context(nc.psum_tensor(name, [128, 512], f32))
        def sem(name):
            return ctx.enter_context(nc.semaphore(name))
        xg_sb = sb("xg_sb", [128, 2, 32, 256]); w1_sb = sb("w1_sb", [128, 2, 32, 64])
        wm2_sb = sb("wm2_sb", [128, 2, 16, 128]); wm3_sb = sb("wm3_sb", [128, 2, 16, 128])
        wf_sb = sb("wf_sb", [128, 2, 16, 16])
        e1_sb = sb("e1_sb", [128, 2, 512]); e2_sb = sb("e2_sb", [128, 2, 512]); e3_sb = sb("e3_sb", [128, 2, 512])
        a1_sb = sb("a1_sb", [128, 2, 512]); a2_sb = sb("a2_sb", [128, 2, 512]); a3_sb = sb("a3_sb", [128, 2, 512])
        b1_sb = sb("b1_sb", [128, 2, 512]); b2_sb = sb("b2_sb", [128, 2, 512]); b3_sb = sb("b3_sb", [128, 2, 512])
        o_sb = sb("o_sb", [128, 2, 256], f32)
        z1a = ps("z1"); z1b = ps("z1b"); z2a = ps("z2"); z2b = ps("z2b")
        z3a = ps("z3"); z3b = ps("z3b")
        zfa = ctx.enter_context(nc.psum_tensor("zfa", [128, 256], f32))
        zfb = ctx.enter_context(nc.psum_tensor("zfb", [128, 256], f32))
        zf = (zfa, zfb)
        w_sems = (sem("w_sem0"), sem("w_sem1"))
        o_sems = (sem("o_sem0"), sem("o_sem1"))
        pe_sem = sem("pe_sem"); act_sem = sem("act_sem")
        dve_sem = sem("dve_sem"); evac_sem = sem("evac_sem")

        z1 = (z1a, z1b)
        z2 = (z2a, z2b)
        z3 = (z3a, z3b)

        def wge(eng, sem, thr):
            if thr > 0:
                eng.wait_ge(sem, thr)

        # ---- two-phase: plan op orders per engine, assign cumulative sem
        # indices, then emit.  Skewed software pipeline: PE iteration t runs
        # L1(t), L2(t-1), L3(t-2), Lf(t-3) so PE never blocks on the
        # ACT->DVE stream chain of the same unit.
        pe_ops, act_ops, dve_ops = [], [], []
        for t in range(NU + 3):
            if t < NU:
                pe_ops.append(("L1", t))
                act_ops.append(("E1", t))
                act_ops.append(("A1", t))
                dve_ops.append(("B1", t))
            if 0 <= t - 1 < NU:
                pe_ops.append(("L2", t - 1))
                act_ops.append(("E2", t - 1))
                dve_ops.append(("A2", t - 1))
                dve_ops.append(("B2", t - 1))
            if 0 <= t - 2 < NU:
                pe_ops.append(("L3", t - 2))
                act_ops.append(("E3", t - 2))
                dve_ops.append(("A3", t - 2))
                dve_ops.append(("B3", t - 2))
            if 0 <= t - 3 < NU:
                pe_ops.append(("Lf", t - 3))
                if (t - 3) % 2 == 1:
                    dve_ops.append(("evac", (t - 3) // 2))
        ipe = {op: n + 1 for n, op in enumerate(pe_ops)}
        iact = {op: n + 1 for n, op in enumerate(act_ops)}
        # evac ops increment evac_sem, not dve_sem -> separate numbering
        idve = {}
        ndve = 0
        for op in dve_ops:
            if op[0] != "evac":
                ndve += 1
                idve[op] = ndve

        def wge(eng, sem, thr):
            if thr > 0:
                eng.wait_ge(sem, thr)

        def wop(eng, sem, table, op):
            if table is iact and "act" in _DISABLE:
                return
            if table is idve and "dve" in _DISABLE:
                return
            if op in table:
                eng.wait_ge(sem, table[op])

        @block.sync
        def _(sync):
            for s in range(NS):
                # super buffers (s % 2) free once super s-2 fully consumed
                wop(sync, pe_sem, ipe, ("Lf", 8 * (s - 1) - 1))
                ws = w_sems[s % 2]
                sync.dma_start(out=xg_sb[:, s % 2], in_=xg_d[s]).then_inc(ws, 16)
                sync.dma_start(out=w1_sb[:, s % 2], in_=w1_d[s]).then_inc(ws, 16)
                sync.dma_start(out=wm2_sb[:, s % 2], in_=wm2_d[s]).then_inc(ws, 16)
                sync.dma_start(out=wm3_sb[:, s % 2], in_=wm3_d[s]).then_inc(ws, 16)
                sync.dma_start(out=wf_sb[:, s % 2], in_=wf_d[s]).then_inc(ws, 16)
                if s >= 1 and "evac" not in _DISABLE and "dve" not in _DISABLE:
                    for v in range(8 * (s - 1) // 2, 8 * s // 2):
                        sync.wait_ge(evac_sem, v + 3)
                        sync.dma_start(out=out_d[v], in_=o_sb[:, v % 2]).then_inc(o_sems[v % 2], 16)
            if "evac" not in _DISABLE and "dve" not in _DISABLE:
                for v in range(8 * (NS - 1) // 2, NW):
                    sync.wait_ge(evac_sem, v + 3)
                    sync.dma_start(out=out_d[v], in_=o_sb[:, v % 2]).then_inc(o_sems[v % 2], 16)
                sync.wait_ge(o_sems[0], 16 * (NW // 2))
                sync.wait_ge(o_sems[1], 16 * (NW // 2))

        @block.tensor
        def _(tensor):
            loaded_super = -1
            for kind, u in pe_ops:
                i = u % 2
                s = u // 8
                j = s % 2
                if kind == "L1":
                    if s > loaded_super:
                        tensor.wait_ge(w_sems[s % 2], 80 * (s // 2 + 1))
                        loaded_super = s
                    wop(tensor, act_sem, iact, ("A1", u - 2))
                    for m in range(4):
                        lp = (4 * u + m) - 32 * s
                        mm = tensor.matmul(
                            z1[i][(m % 2) * 64:(m % 2) * 64 + 64,
                                  (m // 2) * 256:(m // 2) * 256 + 256],
                            w1_sb[:, j, lp, :],
                            xg_sb[:, j, lp, :],
                            start=True, stop=True,
                            tile_position=(0, (m % 2) * 64),
                        )
                    mm.then_inc(pe_sem, 1)
                elif kind == "L2":
                    wop(tensor, act_sem, iact, ("A1", u))
                    wop(tensor, dve_sem, idve, ("B1", u))
                    wop(tensor, act_sem, iact, ("E2", u - 2))
                    wop(tensor, dve_sem, idve, ("A2", u - 2))
                    for g in range(2):
                        ln = (2 * u + g) - 16 * s
                        sl = slice(g * 256, g * 256 + 256)
                        tensor.matmul(z2[i][:, sl], wm2_sb[:, j, ln, :],
                                      a1_sb[:, i, sl], start=True, stop=False)
                        mm = tensor.matmul(z2[i][:, sl], wm2_sb[:, j, ln, :],
                                           b1_sb[:, i, sl], start=False, stop=True)
                    mm.then_inc(pe_sem, 1)
                elif kind == "L3":
                    wop(tensor, dve_sem, idve, ("B2", u))
                    wop(tensor, act_sem, iact, ("E3", u - 2))
                    wop(tensor, dve_sem, idve, ("A3", u - 2))
                    for g in range(2):
                        ln = (2 * u + g) - 16 * s
                        sl = slice(g * 256, g * 256 + 256)
                        tensor.matmul(z3[i][:, sl], wm3_sb[:, j, ln, :],
                                      a2_sb[:, i, sl], start=True, stop=False)
                        mm = tensor.matmul(z3[i][:, sl], wm3_sb[:, j, ln, :],
                                           b2_sb[:, i, sl], start=False, stop=True)
                    mm.then_inc(pe_sem, 1)
                else:  # Lf
                    v = u // 2
                    wop(tensor, dve_sem, idve, ("B3", u))
                    if "dve" not in _DISABLE:
                        # zf[v%2] free: its memset done (+2) and evac(v-2) done
                        wge(tensor, evac_sem, max(v - 1, 0) + 2)
                    for g in range(2):
                        n = 2 * u + g
                        ln = n - 16 * s
                        strip = n % 4
                        sl = slice(g * 256, g * 256 + 256)
                        zout = zf[v % 2][strip * 32:strip * 32 + 16, 0:256]
                        tensor.matmul(zout, wf_sb[:, j, ln, :], a3_sb[:, i, sl],
                                      start=True, stop=False, tile_position=(0, strip * 32))
                        mm = tensor.matmul(zout, wf_sb[:, j, ln, :], b3_sb[:, i, sl],
                                           start=False, stop=True, tile_position=(0, strip * 32))
                    mm.then_inc(pe_sem, 1)

        @block.scalar
        def _(scalar):
            if "act" in _DISABLE:
                return
            for kind, u in act_ops:
                i = u % 2
                if kind == "E1":
                    wop(scalar, pe_sem, ipe, ("L1", u))
                    wop(scalar, dve_sem, idve, ("B1", u - 2))
                    scalar.activation(e1_sb[:, i, :], z1[i][:], Act.Exp,
                                      bias=C0, scale=1.0).then_inc(act_sem, 1)
                elif kind == "A1":
                    scalar.activation(a1_sb[:, i, :], z1[i][:], Act.Relu,
                                      bias=0.0, scale=LAM).then_inc(act_sem, 1)
                elif kind == "E2":
                    wop(scalar, pe_sem, ipe, ("L2", u))
                    wop(scalar, dve_sem, idve, ("B2", u - 2))
                    scalar.activation(e2_sb[:, i, :], z2[i][:], Act.Exp,
                                      bias=C0, scale=1.0).then_inc(act_sem, 1)
                else:  # E3
                    wop(scalar, pe_sem, ipe, ("L3", u))
                    wop(scalar, dve_sem, idve, ("B3", u - 2))
                    scalar.activation(e3_sb[:, i, :], z3[i][:], Act.Exp,
                                      bias=C0, scale=1.0).then_inc(act_sem, 1)

        @block.vector
        def _(vector):
            vector.memset(zfa[:], 0.0).then_inc(evac_sem, 1)
            vector.memset(zfb[:], 0.0).then_inc(evac_sem, 1)
            if "dve" in _DISABLE:
                return
            for kind, u in dve_ops:
                i = u % 2
                if kind == "B1":
                    wop(vector, act_sem, iact, ("E1", u))
                    vector.tensor_scalar(b1_sb[:, i, :], e1_sb[:, i, :], LA, -LA,
                                         Alu.min, Alu.add).then_inc(dve_sem, 1)
                elif kind == "A2":
                    wop(vector, pe_sem, ipe, ("L2", u))
                    wop(vector, act_sem, iact, ("E2", u))
                    vector.tensor_scalar(a2_sb[:, i, :], z2[i][:], 0.0, LAM,
                                         Alu.max, Alu.mult).then_inc(dve_sem, 1)
                elif kind == "B2":
                    wop(vector, act_sem, iact, ("E2", u))
                    vector.tensor_scalar(b2_sb[:, i, :], e2_sb[:, i, :], LA, -LA,
                                         Alu.min, Alu.add).then_inc(dve_sem, 1)
                elif kind == "A3":
                    wop(vector, pe_sem, ipe, ("L3", u))
                    wop(vector, act_sem, iact, ("E3", u))
                    vector.tensor_scalar(a3_sb[:, i, :], z3[i][:], 0.0, LAM,
                                         Alu.max, Alu.mult).then_inc(dve_sem, 1)
                elif kind == "B3":
                    wop(vector, act_sem, iact, ("E3", u))
                    vector.tensor_scalar(b3_sb[:, i, :], e3_sb[:, i, :], LA, -LA,
                                         Alu.min, Alu.add).then_inc(dve_sem, 1)
                else:  # evac, u = window v
                    if "evac" in _DISABLE:
                        continue
                    v = u
                    wop(vector, pe_sem, ipe, ("Lf", 2 * v + 1))
                    wge(vector, o_sems[v % 2], 16 * (v // 2))
                    vector.tensor_copy(
                        o_sb[:, v % 2], zf[v % 2][:],
                    ).then_inc(evac_sem, 1)

    return nc


def _prepare_core_inputs(features, tf_idx, W1, Wm, Wf):
    """Host-side layout: gather + block-diagonal packing, all bf16."""
    fbf = features.astype(BF16)
    maps = []
    for c in range(NCORES):
        g0 = c * GC
        tf_l = np.zeros((GP, K), np.int64)
        tf_l[:GC] = tf_idx[g0:g0 + GC]
        W1_l = np.zeros((GP, W, K), np.float32)
        W1_l[:GC] = W1[g0:g0 + GC]
        Wm_l = np.zeros((D, GP, W, W), np.float32)
        Wm_l[:, :GC] = Wm[:, g0:g0 + GC]
        Wf_l = np.zeros((GP, W), np.float32)
        Wf_l[:GC] = Wf[g0:g0 + GC]

        # xg: [NS, 128, 32, 256]  row q=16j+k of pack p = features[:, tf[8p+j, k]]
        gath = fbf[:, tf_l.reshape(-1)]                     # [B, GP*K] bf16
        xg = np.ascontiguousarray(gath.T).reshape(NP, 128, 256)
        xg = np.ascontiguousarray(
            xg.reshape(NS, 32, 128, 256).transpose(0, 2, 1, 3))

        jj = np.arange(8)
        w1b = np.zeros((NP, 8, K, 8, W), np.float32)
        w1b[:, jj, :, jj, :] = W1_l.reshape(NP, 8, W, K).transpose(
            0, 1, 3, 2).transpose(1, 0, 2, 3)
        w1b = w1b.reshape(NP, 128, 64).astype(BF16)
        w1b = np.ascontiguousarray(
            w1b.reshape(NS, 32, 128, 64).transpose(0, 2, 1, 3))

        j16 = np.arange(16)
        wmb = []
        for l in range(D):
            t = np.zeros((NG, 16, W, 16, W), np.float32)
            t[:, j16, :, j16, :] = Wm_l[l].reshape(NG, 16, W, W).transpose(
                0, 1, 3, 2).transpose(1, 0, 2, 3)
            t = t.reshape(NG, 128, 128).astype(BF16)
            wmb.append(np.ascontiguousarray(
                t.reshape(NS, 16, 128, 128).transpose(0, 2, 1, 3)))

        wfb = np.zeros((NG, 16, W, 16), np.float32)
        wfb[:, j16, :, j16] = Wf_l.reshape(NG, 16, W).transpose(1, 0, 2)
        wfb = wfb.reshape(NG, 128, 16).astype(BF16)
        wfb = np.ascontiguousarray(
            wfb.reshape(NS, 16, 128, 16).transpose(0, 2, 1, 3))

        maps.append({"xg": xg, "w1": w1b, "wm2": wmb[0], "wm3": wmb[1], "wf": wfb})
    return maps


def _assemble(results):
    """Per-core out [NW, 128, 256] -> full [B, G] f32."""
    out = np.empty((B, G), np.float32)
    for c, r in enumerate(results):
        oc = np.asarray(r["out"])                      # [NW, 128, 256]
        # window v, strip g (partitions 32g..32g+16) = genes 16*(4v+g)..+16
        genes = oc.reshape(NW, 4, 32, 256)[:, :, :16, :]   # [NW, 4, 16, 256]
        genes = genes.reshape(GP, 256)[:GC]                # [2500, B]
        out[:, c * GC:(c + 1) * GC] = genes.T
    return out


def kernel(features, tf_idx, W1, b1, Wm, bm, Wf, bf):
    from concourse.bass_utils import run_bass_kernel_spmd

    features = np.asarray(features, np.float32)
    tf_idx = np.asarray(tf_idx)
    assert not np.any(np.asarray(b1)) and not np.any(np.asarray(bm)) \
        and not np.any(np.asarray(bf)), "nonzero biases not supported"

    if "nc" not in _CACHE:
        _CACHE["nc"] = _build()
    nc = _CACHE["nc"]

    in_maps = _prepare_core_inputs(
        features, tf_idx, np.asarray(W1, np.float32),
        np.asarray(Wm, np.float32), np.asarray(Wf, np.float32))

    res = run_bass_kernel_spmd(nc, in_maps, list(range(NCORES)))
    return _assemble(res.results)
